# revision 14
# baseline (speedup 1.0000x reference)
import os
import sys

for _p in ("/opt/trn_rl_repo", "/root/.axon_site/_ro/trn_rl_repo"):
    if os.path.isdir(_p) and _p not in sys.path:
        sys.path.insert(0, _p)

import numpy as np
import ml_dtypes

BF = ml_dtypes.bfloat16

HEADS, D = 12, 64
WINDOW, SHIFT = 16, 1
SCALE = D ** -0.5
B, N, DIM = 2, 2049, 768
INNER = HEADS * D  # 768
TAUG = 258  # CLS slot + tok1/dummy slot + 256 block tokens
NCORES = 8
KT = DIM // 128  # 6
VW = 65  # per-head v stride: 64 value cols + 1 ones col (denominator)

# global token ranges owned by each core (block attention); all starts == 2 mod 16
STARTS = [2, 258, 514, 770, 1026, 1282, 1538, 1794]
ENDS = [258, 514, 770, 1026, 1282, 1538, 1794, 2049]

_NC_CACHE = {}


def _build_nc():
    import concourse.bass as bass
    import concourse.bacc as bacc
    import concourse.mybir as mybir
    import concourse.tile as tile

    f32 = mybir.dt.float32
    bf16 = mybir.dt.bfloat16
    Exp = mybir.ActivationFunctionType.Exp
    Copy = mybir.ActivationFunctionType.Copy

    nc = bacc.Bacc(None, target_bir_lowering=False)

    xaT_ext = nc.declare_dram_parameter("xaT", (B, DIM, TAUG), bf16, isOutput=False)
    wqkv_ext = nc.declare_dram_parameter("w_qkv", (DIM, 3 * INNER), bf16, isOutput=False)
    wout_ext = nc.declare_dram_parameter("w_out", (INNER, DIM), bf16, isOutput=False)
    bout_ext = nc.declare_dram_parameter("b_out", (128, DIM), f32, isOutput=False)
    mask_ext = nc.declare_dram_parameter("masks", (128, 520), bf16, isOutput=False)
    mcls_ext = nc.declare_dram_parameter("mask_cls", (HEADS, TAUG), f32, isOutput=False)
    vspec_ext = nc.declare_dram_parameter("v_spec", (B, HEADS, HEADS * VW), bf16, isOutput=False)
    out_ext = nc.declare_dram_parameter("out_tokens", (B, 256, DIM), f32, isOutput=True)
    cls_ext = nc.declare_dram_parameter("cls_all", (B, VW, HEADS), f32, isOutput=True)

    with tile.TileContext(nc) as tc:
        with (
            tc.tile_pool(name="wpool", bufs=1) as wp,
            tc.tile_pool(name="fpool", bufs=2) as fp,
            tc.tile_pool(name="inpool", bufs=2) as ip,
            tc.tile_pool(name="spool", bufs=3) as sp,
            tc.tile_pool(name="psBig", bufs=2, space="PSUM") as psBig,
            tc.tile_pool(name="psSc", bufs=2, space="PSUM") as psSc,
            tc.tile_pool(name="psAv", bufs=2, space="PSUM") as psAv,
            tc.tile_pool(name="psO", bufs=2, space="PSUM") as psO,
        ):
            # ---- DMA order tuned: batch-0 x first, then w_qkv by column
            # ---- group (k cols first — they gate the head loop), wo/bias last
            # warm the PE clock with dummy matmuls that depend on nothing
            wmsb = wp.tile([128, 128], bf16, tag="wmsb")
            nc.vector.memset(wmsb[:], 0.25)
            wmps = psO.tile([128, 512], f32, tag="o", name="wmps")
            for _ in range(11):
                nc.tensor.matmul(wmps[:, 0:128], wmsb[:], wmsb[:, 0:128], start=True,
                                 stop=True, skip_group_check=True)
            Xw = [None] * B
            t = ip.tile([128, KT * TAUG], bf16, tag="xw", name="xW0")
            nc.sync.dma_start(t[:].rearrange("p (k t) -> p k t", t=TAUG),
                              xaT_ext[0].rearrange("(k p) t -> p k t", p=128))
            Xw[0] = t
            Xb = [[None] * KT for _ in range(B)]
            Xb[0] = [Xw[0][:, k * TAUG:(k + 1) * TAUG] for k in range(KT)]
            mask_t = wp.tile([128, 520], bf16, tag="mask")
            nc.sync.dma_start(mask_t[:], mask_ext[:])
            mclst = wp.tile([HEADS, TAUG], f32, tag="mcls")
            nc.sync.dma_start(mclst[:], mcls_ext[:])
            Ww = wp.tile([128, KT * 3 * INNER], bf16, tag="ww", name="Ww")
            W = [Ww[:, k * 3 * INNER:(k + 1) * 3 * INNER] for k in range(KT)]
            wsrc = wqkv_ext[:].rearrange("(k p) c -> p k c", p=128)
            wdst = Ww[:].rearrange("p (k c) -> p k c", c=3 * INNER)
            for grp in (1, 0, 2):  # k cols, q cols, v cols
                nc.sync.dma_start(wdst[:, :, grp * INNER:(grp + 1) * INNER],
                                  wsrc[:, :, grp * INNER:(grp + 1) * INNER])
            t = ip.tile([128, KT * TAUG], bf16, tag="xw", name="xW1")
            nc.sync.dma_start(t[:].rearrange("p (k t) -> p k t", t=TAUG),
                              xaT_ext[1].rearrange("(k p) t -> p k t", p=128))
            Xw[1] = t
            Xb[1] = [Xw[1][:, k * TAUG:(k + 1) * TAUG] for k in range(KT)]
            bias_t = wp.tile([128, DIM], f32, tag="bias")
            nc.sync.dma_start(bias_t[:], bout_ext[:])
            WOw = wp.tile([128, KT * DIM], bf16, tag="wow", name="WOw")
            nc.sync.dma_start(WOw[:].rearrange("p (k c) -> p k c", c=DIM),
                              wout_ext[:].rearrange("(k p) c -> p k c", p=128))
            WO = [WOw[:, k * DIM:(k + 1) * DIM] for k in range(KT)]

            def proj_phase(b):
                X = Xb[b]
                vspec_t = ip.tile([HEADS, HEADS * VW], bf16, tag="vspec", name=f"vspec{b}")
                nc.sync.dma_start(vspec_t[:], vspec_ext[b])

                QK = [None] * 12
                for fo in [6, 7, 8, 9, 10, 11, 0, 1, 2, 3, 4, 5]:
                    P = psBig.tile([128, TAUG], f32, tag="big")
                    for fi in range(KT):
                        nc.tensor.matmul(P[:], W[fi][:, fo * 128:(fo + 1) * 128], X[fi][:],
                                         start=(fi == 0), stop=(fi == KT - 1))
                    t = fp.tile([128, TAUG], bf16, tag=f"qk{fo}", name=f"qk{b}_{fo}")
                    nc.vector.tensor_copy(t[:], P[:])
                    QK[fo] = t

                kbd = fp.tile([128, KT * HEADS], bf16, tag="kbd", name=f"kbd{b}")
                nc.vector.memset(kbd[:], 0.0)
                for h in range(HEADS):
                    r0 = (h % 2) * 64
                    c = (h // 2) * HEADS + h
                    nc.vector.tensor_copy(kbd[r0:r0 + 64, c:c + 1],
                                          QK[6 + h // 2][r0:r0 + 64, 0:1])
                E = psAv.tile([HEADS, TAUG], f32, tag="av", name=f"E{b}")
                for i in range(KT):
                    nc.tensor.matmul(E[:], kbd[:, i * HEADS:(i + 1) * HEADS], QK[i][:],
                                     start=(i == 0), stop=(i == KT - 1))
                esb = sp.tile([HEADS, TAUG], f32, tag="esb", name=f"esb{b}")
                nc.scalar.activation(esb[:], E[:], Exp, scale=SCALE)
                esb_bf = fp.tile([HEADS, TAUG], bf16, tag="esb_bf", name=f"esbbf{b}")
                nc.vector.tensor_mul(esb_bf[:], esb[:], mclst[:])

                Vsb = []
                for s in range(2):
                    vt = fp.tile([128, HEADS * VW], bf16, tag=f"v{s}", name=f"v{b}_{s}")
                    for half in range(2):
                        c0, cw, nh = (0, 512, 8) if half == 0 else (512, 256, 4)
                        P = psBig.tile([128, cw], f32, tag="big")
                        for fi in range(KT):
                            nc.tensor.matmul(P[:], X[fi][:, 2 + 128 * s: 2 + 128 * (s + 1)],
                                             W[fi][:, 2 * INNER + c0: 2 * INNER + c0 + cw],
                                             start=(fi == 0), stop=(fi == KT - 1))
                        dst = vt[:, half * 8 * VW: half * 8 * VW + nh * VW]
                        dst = dst.rearrange("p (h c) -> p h c", c=VW)[:, :, 0:64]
                        src2 = P[:].rearrange("p (h c) -> p h c", c=64)
                        nc.vector.tensor_copy(dst, src2)
                    nc.vector.memset(vt[:, 64:HEADS * VW:VW], 1.0)
                    Vsb.append(vt)

                aTfm = [fp.tile([128, TAUG], bf16, tag=f"a{i}", name=f"aTfm{b}_{i}")
                        for i in range(KT)]
                cls_all = sp.tile([VW, HEADS], f32, tag="cls_all", name=f"cls{b}")
                return dict(vspec=vspec_t, QK=QK, esb_bf=esb_bf, Vsb=Vsb,
                            aTfm=aTfm, cls_all=cls_all, O=[None, None])

            def head_pair(b, st, j):
                QK, Vsb, esb_bf, vspec_t = st["QK"], st["Vsb"], st["esb_bf"], st["vspec"]
                aTfm, cls_all = st["aTfm"], st["cls_all"]
                ha, hb2 = 2 * j, 2 * j + 1
                qa, ka = QK[j][0:64, :], QK[6 + j][0:64, :]
                qb, kb = QK[j][64:128, :], QK[6 + j][64:128, :]

                Ba = psAv.tile([VW, TAUG], f32, tag="av", name=f"Ba{b}_{j}")
                Bb = psAv.tile([VW, TAUG], f32, tag="av", name=f"Bb{b}_{j}")
                nc.tensor.matmul(Ba[:], vspec_t[:, ha * VW:(ha + 1) * VW], esb_bf[:],
                                 start=True, stop=False)
                nc.tensor.matmul(Bb[:], vspec_t[:, hb2 * VW:(hb2 + 1) * VW], esb_bf[:],
                                 start=True, stop=False)
                Aa = psSc.tile([128, 260], f32, tag="sc", name=f"Aa{b}_{j}")
                Ab = psSc.tile([128, 260], f32, tag="sc", name=f"Ab{b}_{j}")
                nc.tensor.matmul(Aa[:, 0:130], ka[:, 2:130], qa[:, 0:130],
                                 start=True, stop=True)
                nc.tensor.matmul(Ab[:, 0:130], kb[:, 2:130], qb[:, 0:130],
                                 start=True, stop=True)
                nc.tensor.matmul(Aa[:, 130:258], ka[:, 130:258], qa[:, 130:258],
                                 start=True, stop=True, skip_group_check=True)
                nc.tensor.matmul(Ab[:, 130:258], kb[:, 130:258], qb[:, 130:258],
                                 start=True, stop=True, skip_group_check=True)
                nc.tensor.matmul(Aa[:, 258:259], ka[:, 130:258], qa[:, 0:1],
                                 start=True, stop=True, skip_group_check=True)
                nc.tensor.matmul(Ab[:, 258:259], kb[:, 130:258], qb[:, 0:1],
                                 start=True, stop=True, skip_group_check=True)
                proba = sp.tile([128, 260], bf16, tag="proba")
                probb = sp.tile([128, 260], bf16, tag="probb")
                nc.scalar.activation(proba[:, 0:259], Aa[:, 0:259], Exp, scale=SCALE)
                nc.scalar.activation(probb[:, 0:259], Ab[:, 0:259], Exp, scale=SCALE)
                nc.vector.tensor_mul(proba[:, 0:259], proba[:, 0:259], mask_t[:, 0:259])
                nc.vector.tensor_mul(probb[:, 0:259], probb[:, 0:259], mask_t[:, 0:259])
                for Bx, px, h in ((Ba, proba, ha), (Bb, probb, hb2)):
                    nc.tensor.matmul(Bx[:, 0:130], Vsb[0][:, h * VW:(h + 1) * VW],
                                     px[:, 0:130], start=False, stop=False)
                    nc.tensor.matmul(Bx[:, 130:258], Vsb[1][:, h * VW:(h + 1) * VW],
                                     px[:, 130:258], start=False, stop=False)
                    nc.tensor.matmul(Bx[:, 0:1], Vsb[1][:, h * VW:(h + 1) * VW],
                                     px[:, 258:259], start=False, stop=True)
                for Bx, h, r0 in ((Ba, ha, 0), (Bb, hb2, 64)):
                    dsb = sp.tile([1, TAUG], f32, tag="dsb")
                    nc.vector.tensor_copy(dsb[:], Bx[64:65, :])
                    rec = sp.tile([1, TAUG], f32, tag="rec")
                    nc.vector.reciprocal_approx_fast(rec[:], dsb[:])
                    recb = sp.tile([D, TAUG], f32, tag="recb")
                    nc.gpsimd.partition_broadcast(recb[:], rec[:])
                    nc.vector.tensor_mul(aTfm[j][r0:r0 + 64, :], Bx[0:64, :], recb[:])
                    nc.vector.tensor_copy(cls_all[:, h:h + 1], Bx[:, 0:1])
                # out-proj tok-tile 0 accumulates feature tile j as it completes
                if j == 0:
                    st["O"][0] = psO.tile([128, 512], f32, tag="o", name=f"O0_{b}")
                    st["O"][1] = psO.tile([128, 256], f32, tag="o", name=f"O1_{b}")
                nc.tensor.matmul(st["O"][0][:], aTfm[j][:, 2:130], WO[j][:, 0:512],
                                 start=(j == 0), stop=(j == KT - 1))
                nc.tensor.matmul(st["O"][1][:], aTfm[j][:, 2:130], WO[j][:, 512:768],
                                 start=(j == 0), stop=(j == KT - 1))

            def finish_phase(b, st):
                nc.sync.dma_start(cls_ext[b], st["cls_all"][:])
                aTfm = st["aTfm"]
                osb0 = ip.tile([128, DIM], f32, tag="osb0", name=f"osb0_{b}")
                nc.vector.tensor_add(osb0[:, 0:512], st["O"][0][:], bias_t[:, 0:512])
                nc.vector.tensor_add(osb0[:, 512:768], st["O"][1][:], bias_t[:, 512:768])
                nc.sync.dma_start(out_ext[b, 0:128, :], osb0[:])
                osb1 = ip.tile([128, DIM], f32, tag="osb1", name=f"osb1_{b}")
                if "O1t" in st:
                    nc.vector.tensor_add(osb1[:, 0:512], st["O1t"][0][:], bias_t[:, 0:512])
                    nc.vector.tensor_add(osb1[:, 512:768], st["O1t"][1][:], bias_t[:, 512:768])
                else:
                    for half in range(2):
                        f0, fw = (0, 512) if half == 0 else (512, 256)
                        PO = psO.tile([128, fw], f32, tag="o", name=f"PO{b}_{half}")
                        for i in range(KT):
                            nc.tensor.matmul(PO[:], aTfm[i][:, 130:258], WO[i][:, f0:f0 + fw],
                                             start=(i == 0), stop=(i == KT - 1))
                        nc.vector.tensor_add(osb1[:, f0:f0 + fw], PO[:], bias_t[:, f0:f0 + fw])
                nc.sync.dma_start(out_ext[b, 128:256, :], osb1[:])

            for b in range(B):
                st = proj_phase(b)
                for j in range(KT):
                    head_pair(b, st, j)
                finish_phase(b, st)

    nc.compile()
    return nc


def _get_nc():
    if "nc" not in _NC_CACHE:
        _NC_CACHE["nc"] = _build_nc()
    return _NC_CACHE["nc"]


def _make_masks(core):
    start = STARTS[core]
    m = np.zeros((128, 260), dtype=np.float32)
    _fill_mask(m, core)
    return np.concatenate([m, m], axis=1)  # [128, 520] for the pair-merged mul


def _fill_mask(m, core):
    start = STARTS[core]
    for s in range(2):
        g = start + s * 128 + np.arange(128)  # global token ids of this stripe's keys
        real = g < 2049
        blk = (g - 2) // 16
        bd = (blk[:, None] == blk[None, :]) & real[:, None] & real[None, :]
        if s == 0:
            m[:, 0] = real.astype(np.float32)  # CLS query attends all real s0 keys
            m[:, 2:130] = bd
        else:
            m[:, 130:258] = bd
            m[:, 258] = real.astype(np.float32)  # CLS query col vs s1 keys


def make_in_maps(x, w_qkv, w_out, b_out):
    x = np.asarray(x, dtype=np.float32)
    w_qkv = np.asarray(w_qkv, dtype=np.float32)
    w_out = np.asarray(w_out, dtype=np.float32)
    b_out = np.asarray(b_out, dtype=np.float32)
    w_v = w_qkv[:, 2 * INNER:]
    wqkv_bf = w_qkv.astype(BF)
    wout_bf = w_out.astype(BF)
    bias128 = np.tile(b_out.reshape(1, DIM), (128, 1)).astype(np.float32)
    in_maps = []
    for c in range(NCORES):
        xa = np.zeros((B, TAUG, DIM), dtype=np.float32)
        xa[:, 0, :] = x[:, 0, :]
        if c == 0:
            xa[:, 1, :] = x[:, 1, :]
        L = ENDS[c] - STARTS[c]
        xa[:, 2:2 + L, :] = x[:, STARTS[c]:ENDS[c], :]
        xaT = np.ascontiguousarray(xa.transpose(0, 2, 1)).astype(BF)
        mask_cls = np.ones((HEADS, TAUG), dtype=np.float32)
        if c > 0:
            mask_cls[:, 0] = 0.0
        v_cls = xa[:, 0, :] @ w_v  # [B, 768]
        v_spec = np.zeros((B, HEADS, HEADS * VW), dtype=np.float32)
        for h in range(HEADS):
            v_spec[:, h, h * VW:h * VW + 64] = v_cls[:, h * 64:(h + 1) * 64]
            v_spec[:, h, h * VW + 64] = 1.0
        in_maps.append({
            "xaT": xaT,
            "w_qkv": wqkv_bf,
            "w_out": wout_bf,
            "b_out": bias128,
            "masks": _make_masks(c).astype(BF),
            "mask_cls": mask_cls,
            "v_spec": v_spec.astype(BF),
        })
    return in_maps


def kernel(x, w_qkv, w_out, b_out):
    x = np.asarray(x, dtype=np.float32)
    w_qkv = np.asarray(w_qkv, dtype=np.float32)
    w_out = np.asarray(w_out, dtype=np.float32)
    b_out = np.asarray(b_out, dtype=np.float32)

    in_maps = make_in_maps(x, w_qkv, w_out, b_out)

    from concourse.bass_utils import run_bass_kernel_spmd

    nc = _get_nc()
    res = run_bass_kernel_spmd(nc, in_maps, core_ids=list(range(NCORES))).results

    out = np.empty((B, N, DIM), dtype=np.float32)
    for c in range(NCORES):
        L = ENDS[c] - STARTS[c]
        out[:, STARTS[c]:ENDS[c], :] = res[c]["out_tokens"][:, :L, :]

    # host-side special rows (CLS = global token 0, tok1 = global token 1)
    w_q = w_qkv[:, :INNER].astype(np.float64)
    w_k = w_qkv[:, INNER:2 * INNER].astype(np.float64)
    w_v = w_qkv[:, 2 * INNER:].astype(np.float64)
    w_o = w_out.astype(np.float64)
    b_o = b_out.astype(np.float64)
    for b in range(B):
        x0 = x[b, 0].astype(np.float64)
        x1 = x[b, 1].astype(np.float64)
        q_cls = (x0 @ w_q).reshape(HEADS, D)
        q_t1 = (x1 @ w_q).reshape(HEADS, D)
        k_cls = (x0 @ w_k).reshape(HEADS, D)
        k_t1 = (x1 @ w_k).reshape(HEADS, D)
        v_cls = (x0 @ w_v).reshape(HEADS, D)
        v_t1 = (x1 @ w_v).reshape(HEADS, D)

        # CLS row: sum device partials over cores, add tok1-as-key term on host
        num = np.zeros((D, HEADS), dtype=np.float64)
        den = np.zeros(HEADS, dtype=np.float64)
        for c in range(NCORES):
            cl = res[c]["cls_all"][b].astype(np.float64)
            num += cl[0:D, :]
            den += cl[D, :]
        e_t1 = np.exp(SCALE * (q_cls * k_t1).sum(1))  # [HEADS]
        num += v_t1.T * e_t1[None, :]
        den += e_t1
        att = (num / den[None, :]).T.reshape(INNER)  # f = h*64 + d
        out[b, 0, :] = (att @ w_o + b_o).astype(np.float32)

        # tok1 row: attends {CLS, itself} only — fully host-computed
        e_c = np.exp(SCALE * (q_t1 * k_cls).sum(1))
        e_1 = np.exp(SCALE * (q_t1 * k_t1).sum(1))
        att1 = ((e_c[:, None] * v_cls + e_1[:, None] * v_t1)
                / (e_c + e_1)[:, None]).reshape(INNER)
        out[b, 1, :] = (att1 @ w_o + b_o).astype(np.float32)
    return out


# revision 15
# speedup vs baseline: 1.1510x; 1.1510x over previous
import os
import sys

for _p in ("/opt/trn_rl_repo", "/root/.axon_site/_ro/trn_rl_repo"):
    if os.path.isdir(_p) and _p not in sys.path:
        sys.path.insert(0, _p)

import numpy as np
import ml_dtypes

BF = ml_dtypes.bfloat16

HEADS, D = 12, 64
WINDOW, SHIFT = 16, 1
SCALE = D ** -0.5
B, N, DIM = 2, 2049, 768
INNER = HEADS * D  # 768
TAUG = 258  # CLS slot + tok1/dummy slot + 256 block tokens
NCORES = 8
KT = DIM // 128  # 6
VW = 65  # per-head v stride: 64 value cols + 1 ones col (denominator)

# global token ranges owned by each core (block attention); all starts == 2 mod 16
STARTS = [2, 258, 514, 770, 1026, 1282, 1538, 1794]
ENDS = [258, 514, 770, 1026, 1282, 1538, 1794, 2049]

_NC_CACHE = {}


def _build_nc():
    import concourse.bass as bass
    import concourse.bacc as bacc
    import concourse.mybir as mybir
    import concourse.tile as tile

    f32 = mybir.dt.float32
    bf16 = mybir.dt.bfloat16
    Exp = mybir.ActivationFunctionType.Exp
    Copy = mybir.ActivationFunctionType.Copy

    nc = bacc.Bacc(None, target_bir_lowering=False)

    xaT_ext = nc.declare_dram_parameter("xaT", (B, DIM, TAUG), bf16, isOutput=False)
    wqkv_ext = nc.declare_dram_parameter("w_qkv", (DIM, 3 * INNER), bf16, isOutput=False)
    wout_ext = nc.declare_dram_parameter("w_out", (INNER, DIM), bf16, isOutput=False)
    bout_ext = nc.declare_dram_parameter("b_out", (128, DIM), f32, isOutput=False)
    mask_ext = nc.declare_dram_parameter("masks", (128, 520), bf16, isOutput=False)
    mcls_ext = nc.declare_dram_parameter("mask_cls", (HEADS, TAUG), f32, isOutput=False)
    vspec_ext = nc.declare_dram_parameter("v_spec", (B, HEADS, HEADS * VW), bf16, isOutput=False)
    out_ext = nc.declare_dram_parameter("out_tokens", (B, 256, DIM), f32, isOutput=True)
    cls_ext = nc.declare_dram_parameter("cls_all", (B, VW, HEADS), f32, isOutput=True)

    with tile.TileContext(nc) as tc:
        with (
            tc.tile_pool(name="wpool", bufs=1) as wp,
            tc.tile_pool(name="fpool", bufs=2) as fp,
            tc.tile_pool(name="inpool", bufs=2) as ip,
            tc.tile_pool(name="spool", bufs=3) as sp,
            tc.tile_pool(name="psBig", bufs=2, space="PSUM") as psBig,
            tc.tile_pool(name="psSc", bufs=2, space="PSUM") as psSc,
            tc.tile_pool(name="psAv", bufs=2, space="PSUM") as psAv,
            tc.tile_pool(name="psO", bufs=2, space="PSUM") as psO,
        ):
            # ---- DMA order tuned: batch-0 x first, then w_qkv by column
            # ---- group (k cols first — they gate the head loop), wo/bias last
            # warm the PE clock with dummy matmuls that depend on nothing
            wmsb = wp.tile([128, 128], bf16, tag="wmsb")
            nc.vector.memset(wmsb[:], 0.25)
            wmps = psO.tile([128, 512], f32, tag="o", name="wmps")
            for _ in range(11):
                nc.tensor.matmul(wmps[:, 0:128], wmsb[:], wmsb[:, 0:128], start=True,
                                 stop=True, skip_group_check=True)
            Xw = [None] * B
            t = ip.tile([128, KT * TAUG], bf16, tag="xw", name="xW0")
            nc.sync.dma_start(t[:].rearrange("p (k t) -> p k t", t=TAUG),
                              xaT_ext[0].rearrange("(k p) t -> p k t", p=128))
            Xw[0] = t
            Xb = [[None] * KT for _ in range(B)]
            Xb[0] = [Xw[0][:, k * TAUG:(k + 1) * TAUG] for k in range(KT)]
            mask_t = wp.tile([128, 520], bf16, tag="mask")
            nc.sync.dma_start(mask_t[:], mask_ext[:])
            mclst = wp.tile([HEADS, TAUG], f32, tag="mcls")
            nc.sync.dma_start(mclst[:], mcls_ext[:])
            Ww = wp.tile([128, KT * 3 * INNER], bf16, tag="ww", name="Ww")
            W = [Ww[:, k * 3 * INNER:(k + 1) * 3 * INNER] for k in range(KT)]
            wsrc = wqkv_ext[:].rearrange("(k p) c -> p k c", p=128)
            wdst = Ww[:].rearrange("p (k c) -> p k c", c=3 * INNER)
            for grp in (1, 0, 2):  # k cols, q cols, v cols
                nc.sync.dma_start(wdst[:, :, grp * INNER:(grp + 1) * INNER],
                                  wsrc[:, :, grp * INNER:(grp + 1) * INNER])
            t = ip.tile([128, KT * TAUG], bf16, tag="xw", name="xW1")
            nc.sync.dma_start(t[:].rearrange("p (k t) -> p k t", t=TAUG),
                              xaT_ext[1].rearrange("(k p) t -> p k t", p=128))
            Xw[1] = t
            Xb[1] = [Xw[1][:, k * TAUG:(k + 1) * TAUG] for k in range(KT)]
            bias_t = wp.tile([128, DIM], f32, tag="bias")
            nc.sync.dma_start(bias_t[:], bout_ext[:])
            WOw = wp.tile([128, KT * DIM], bf16, tag="wow", name="WOw")
            nc.sync.dma_start(WOw[:].rearrange("p (k c) -> p k c", c=DIM),
                              wout_ext[:].rearrange("(k p) c -> p k c", p=128))
            WO = [WOw[:, k * DIM:(k + 1) * DIM] for k in range(KT)]

            def proj_phase(b):
                X = Xb[b]
                vspec_t = ip.tile([HEADS, HEADS * VW], bf16, tag="vspec", name=f"vspec{b}")
                nc.sync.dma_start(vspec_t[:], vspec_ext[b])

                QK = [None] * 12
                for fo in [6, 7, 8, 9, 10, 11, 0, 1, 2, 3, 4, 5]:
                    P = psBig.tile([128, TAUG], f32, tag="big")
                    for fi in range(KT):
                        nc.tensor.matmul(P[:], W[fi][:, fo * 128:(fo + 1) * 128], X[fi][:],
                                         start=(fi == 0), stop=(fi == KT - 1))
                    t = fp.tile([128, TAUG], bf16, tag=f"qk{fo}", name=f"qk{b}_{fo}")
                    nc.vector.tensor_copy(t[:], P[:])
                    QK[fo] = t

                kbd = fp.tile([128, KT * HEADS], bf16, tag="kbd", name=f"kbd{b}")
                nc.vector.memset(kbd[:], 0.0)
                for h in range(HEADS):
                    r0 = (h % 2) * 64
                    c = (h // 2) * HEADS + h
                    nc.vector.tensor_copy(kbd[r0:r0 + 64, c:c + 1],
                                          QK[6 + h // 2][r0:r0 + 64, 0:1])
                E = psAv.tile([HEADS, TAUG], f32, tag="av", name=f"E{b}")
                for i in range(KT):
                    nc.tensor.matmul(E[:], kbd[:, i * HEADS:(i + 1) * HEADS], QK[i][:],
                                     start=(i == 0), stop=(i == KT - 1))
                esb = sp.tile([HEADS, TAUG], f32, tag="esb", name=f"esb{b}")
                nc.scalar.activation(esb[:], E[:], Exp, scale=SCALE)
                esb_bf = fp.tile([HEADS, TAUG], bf16, tag="esb_bf", name=f"esbbf{b}")
                nc.vector.tensor_mul(esb_bf[:], esb[:], mclst[:])

                Vsb = []
                for s in range(2):
                    vt = fp.tile([128, HEADS * VW], bf16, tag=f"v{s}", name=f"v{b}_{s}")
                    for half in range(2):
                        c0, cw, nh = (0, 512, 8) if half == 0 else (512, 256, 4)
                        P = psBig.tile([128, cw], f32, tag="big")
                        for fi in range(KT):
                            nc.tensor.matmul(P[:], X[fi][:, 2 + 128 * s: 2 + 128 * (s + 1)],
                                             W[fi][:, 2 * INNER + c0: 2 * INNER + c0 + cw],
                                             start=(fi == 0), stop=(fi == KT - 1))
                        dst = vt[:, half * 8 * VW: half * 8 * VW + nh * VW]
                        dst = dst.rearrange("p (h c) -> p h c", c=VW)[:, :, 0:64]
                        src2 = P[:].rearrange("p (h c) -> p h c", c=64)
                        nc.vector.tensor_copy(dst, src2)
                    nc.vector.memset(vt[:, 64:HEADS * VW:VW], 1.0)
                    Vsb.append(vt)

                aTfm = [fp.tile([128, TAUG], bf16, tag=f"a{i}", name=f"aTfm{b}_{i}")
                        for i in range(KT)]
                cls_all = sp.tile([VW, HEADS], f32, tag="cls_all", name=f"cls{b}")
                return dict(vspec=vspec_t, QK=QK, esb_bf=esb_bf, Vsb=Vsb,
                            aTfm=aTfm, cls_all=cls_all, O=[None, None])

            def head_pair(b, st, j):
                QK, Vsb, esb_bf, vspec_t = st["QK"], st["Vsb"], st["esb_bf"], st["vspec"]
                aTfm, cls_all = st["aTfm"], st["cls_all"]
                ha, hb2 = 2 * j, 2 * j + 1
                qa, ka = QK[j][0:64, :], QK[6 + j][0:64, :]
                qb, kb = QK[j][64:128, :], QK[6 + j][64:128, :]

                Ba = psAv.tile([VW, TAUG], f32, tag="av", name=f"Ba{b}_{j}")
                Bb = psAv.tile([VW, TAUG], f32, tag="av", name=f"Bb{b}_{j}")
                nc.tensor.matmul(Ba[:], vspec_t[:, ha * VW:(ha + 1) * VW], esb_bf[:],
                                 start=True, stop=False)
                nc.tensor.matmul(Bb[:], vspec_t[:, hb2 * VW:(hb2 + 1) * VW], esb_bf[:],
                                 start=True, stop=False)
                Aa = psSc.tile([128, 260], f32, tag="sc", name=f"Aa{b}_{j}")
                Ab = psSc.tile([128, 260], f32, tag="sc", name=f"Ab{b}_{j}")
                nc.tensor.matmul(Aa[:, 0:130], ka[:, 2:130], qa[:, 0:130],
                                 start=True, stop=True)
                nc.tensor.matmul(Ab[:, 0:130], kb[:, 2:130], qb[:, 0:130],
                                 start=True, stop=True)
                nc.tensor.matmul(Aa[:, 130:258], ka[:, 130:258], qa[:, 130:258],
                                 start=True, stop=True, skip_group_check=True)
                nc.tensor.matmul(Ab[:, 130:258], kb[:, 130:258], qb[:, 130:258],
                                 start=True, stop=True, skip_group_check=True)
                nc.tensor.matmul(Aa[:, 258:259], ka[:, 130:258], qa[:, 0:1],
                                 start=True, stop=True, skip_group_check=True)
                nc.tensor.matmul(Ab[:, 258:259], kb[:, 130:258], qb[:, 0:1],
                                 start=True, stop=True, skip_group_check=True)
                proba = sp.tile([128, 260], bf16, tag="proba")
                probb = sp.tile([128, 260], bf16, tag="probb")
                nc.scalar.activation(proba[:, 0:259], Aa[:, 0:259], Exp, scale=SCALE)
                nc.scalar.activation(probb[:, 0:259], Ab[:, 0:259], Exp, scale=SCALE)
                nc.vector.tensor_mul(proba[:, 0:259], proba[:, 0:259], mask_t[:, 0:259])
                nc.vector.tensor_mul(probb[:, 0:259], probb[:, 0:259], mask_t[:, 0:259])
                for Bx, px, h in ((Ba, proba, ha), (Bb, probb, hb2)):
                    nc.tensor.matmul(Bx[:, 0:130], Vsb[0][:, h * VW:(h + 1) * VW],
                                     px[:, 0:130], start=False, stop=False)
                    nc.tensor.matmul(Bx[:, 130:258], Vsb[1][:, h * VW:(h + 1) * VW],
                                     px[:, 130:258], start=False, stop=False)
                    nc.tensor.matmul(Bx[:, 0:1], Vsb[1][:, h * VW:(h + 1) * VW],
                                     px[:, 258:259], start=False, stop=True)
                for Bx, h, r0 in ((Ba, ha, 0), (Bb, hb2, 64)):
                    dsb = sp.tile([1, TAUG], f32, tag="dsb")
                    nc.scalar.activation(dsb[:], Bx[64:65, :], Copy)
                    rec = sp.tile([1, TAUG], f32, tag="rec")
                    nc.vector.reciprocal_approx_fast(rec[:], dsb[:])
                    recb = sp.tile([D, TAUG], f32, tag="recb")
                    nc.gpsimd.partition_broadcast(recb[:], rec[:])
                    nc.vector.tensor_mul(aTfm[j][r0:r0 + 64, :], Bx[0:64, :], recb[:])
                    nc.vector.tensor_copy(cls_all[:, h:h + 1], Bx[:, 0:1])
                # out-proj tok-tile 0 accumulates feature tile j as it completes
                if j == 0:
                    st["O"][0] = psO.tile([128, 512], f32, tag="o", name=f"O0_{b}")
                    st["O"][1] = psO.tile([128, 256], f32, tag="o", name=f"O1_{b}")
                nc.tensor.matmul(st["O"][0][:], aTfm[j][:, 2:130], WO[j][:, 0:512],
                                 start=(j == 0), stop=(j == KT - 1))
                nc.tensor.matmul(st["O"][1][:], aTfm[j][:, 2:130], WO[j][:, 512:768],
                                 start=(j == 0), stop=(j == KT - 1))

            def finish_phase(b, st):
                nc.sync.dma_start(cls_ext[b], st["cls_all"][:])
                aTfm = st["aTfm"]
                osb0 = ip.tile([128, DIM], f32, tag="osb0", name=f"osb0_{b}")
                nc.vector.tensor_add(osb0[:, 0:512], st["O"][0][:], bias_t[:, 0:512])
                nc.vector.tensor_add(osb0[:, 512:768], st["O"][1][:], bias_t[:, 512:768])
                nc.sync.dma_start(out_ext[b, 0:128, :], osb0[:])
                osb1 = ip.tile([128, DIM], f32, tag="osb1", name=f"osb1_{b}")
                if "O1t" in st:
                    nc.vector.tensor_add(osb1[:, 0:512], st["O1t"][0][:], bias_t[:, 0:512])
                    nc.vector.tensor_add(osb1[:, 512:768], st["O1t"][1][:], bias_t[:, 512:768])
                else:
                    for half in range(2):
                        f0, fw = (0, 512) if half == 0 else (512, 256)
                        PO = psO.tile([128, fw], f32, tag="o", name=f"PO{b}_{half}")
                        for i in range(KT):
                            nc.tensor.matmul(PO[:], aTfm[i][:, 130:258], WO[i][:, f0:f0 + fw],
                                             start=(i == 0), stop=(i == KT - 1))
                        nc.vector.tensor_add(osb1[:, f0:f0 + fw], PO[:], bias_t[:, f0:f0 + fw])
                nc.sync.dma_start(out_ext[b, 128:256, :], osb1[:])

            for b in range(B):
                st = proj_phase(b)
                for j in range(KT):
                    head_pair(b, st, j)
                finish_phase(b, st)

    nc.compile()
    return nc


def _get_nc():
    if "nc" not in _NC_CACHE:
        _NC_CACHE["nc"] = _build_nc()
    return _NC_CACHE["nc"]


def _make_masks(core):
    start = STARTS[core]
    m = np.zeros((128, 260), dtype=np.float32)
    _fill_mask(m, core)
    return np.concatenate([m, m], axis=1)  # [128, 520] for the pair-merged mul


def _fill_mask(m, core):
    start = STARTS[core]
    for s in range(2):
        g = start + s * 128 + np.arange(128)  # global token ids of this stripe's keys
        real = g < 2049
        blk = (g - 2) // 16
        bd = (blk[:, None] == blk[None, :]) & real[:, None] & real[None, :]
        if s == 0:
            m[:, 0] = real.astype(np.float32)  # CLS query attends all real s0 keys
            m[:, 2:130] = bd
        else:
            m[:, 130:258] = bd
            m[:, 258] = real.astype(np.float32)  # CLS query col vs s1 keys


def make_in_maps(x, w_qkv, w_out, b_out):
    x = np.asarray(x, dtype=np.float32)
    w_qkv = np.asarray(w_qkv, dtype=np.float32)
    w_out = np.asarray(w_out, dtype=np.float32)
    b_out = np.asarray(b_out, dtype=np.float32)
    w_v = w_qkv[:, 2 * INNER:]
    wqkv_bf = w_qkv.astype(BF)
    wout_bf = w_out.astype(BF)
    bias128 = np.tile(b_out.reshape(1, DIM), (128, 1)).astype(np.float32)
    in_maps = []
    for c in range(NCORES):
        xa = np.zeros((B, TAUG, DIM), dtype=np.float32)
        xa[:, 0, :] = x[:, 0, :]
        if c == 0:
            xa[:, 1, :] = x[:, 1, :]
        L = ENDS[c] - STARTS[c]
        xa[:, 2:2 + L, :] = x[:, STARTS[c]:ENDS[c], :]
        xaT = np.ascontiguousarray(xa.transpose(0, 2, 1)).astype(BF)
        mask_cls = np.ones((HEADS, TAUG), dtype=np.float32)
        if c > 0:
            mask_cls[:, 0] = 0.0
        v_cls = xa[:, 0, :] @ w_v  # [B, 768]
        v_spec = np.zeros((B, HEADS, HEADS * VW), dtype=np.float32)
        for h in range(HEADS):
            v_spec[:, h, h * VW:h * VW + 64] = v_cls[:, h * 64:(h + 1) * 64]
            v_spec[:, h, h * VW + 64] = 1.0
        in_maps.append({
            "xaT": xaT,
            "w_qkv": wqkv_bf,
            "w_out": wout_bf,
            "b_out": bias128,
            "masks": _make_masks(c).astype(BF),
            "mask_cls": mask_cls,
            "v_spec": v_spec.astype(BF),
        })
    return in_maps


def kernel(x, w_qkv, w_out, b_out):
    x = np.asarray(x, dtype=np.float32)
    w_qkv = np.asarray(w_qkv, dtype=np.float32)
    w_out = np.asarray(w_out, dtype=np.float32)
    b_out = np.asarray(b_out, dtype=np.float32)

    in_maps = make_in_maps(x, w_qkv, w_out, b_out)

    from concourse.bass_utils import run_bass_kernel_spmd

    nc = _get_nc()
    res = run_bass_kernel_spmd(nc, in_maps, core_ids=list(range(NCORES))).results

    out = np.empty((B, N, DIM), dtype=np.float32)
    for c in range(NCORES):
        L = ENDS[c] - STARTS[c]
        out[:, STARTS[c]:ENDS[c], :] = res[c]["out_tokens"][:, :L, :]

    # host-side special rows (CLS = global token 0, tok1 = global token 1)
    w_q = w_qkv[:, :INNER].astype(np.float64)
    w_k = w_qkv[:, INNER:2 * INNER].astype(np.float64)
    w_v = w_qkv[:, 2 * INNER:].astype(np.float64)
    w_o = w_out.astype(np.float64)
    b_o = b_out.astype(np.float64)
    for b in range(B):
        x0 = x[b, 0].astype(np.float64)
        x1 = x[b, 1].astype(np.float64)
        q_cls = (x0 @ w_q).reshape(HEADS, D)
        q_t1 = (x1 @ w_q).reshape(HEADS, D)
        k_cls = (x0 @ w_k).reshape(HEADS, D)
        k_t1 = (x1 @ w_k).reshape(HEADS, D)
        v_cls = (x0 @ w_v).reshape(HEADS, D)
        v_t1 = (x1 @ w_v).reshape(HEADS, D)

        # CLS row: sum device partials over cores, add tok1-as-key term on host
        num = np.zeros((D, HEADS), dtype=np.float64)
        den = np.zeros(HEADS, dtype=np.float64)
        for c in range(NCORES):
            cl = res[c]["cls_all"][b].astype(np.float64)
            num += cl[0:D, :]
            den += cl[D, :]
        e_t1 = np.exp(SCALE * (q_cls * k_t1).sum(1))  # [HEADS]
        num += v_t1.T * e_t1[None, :]
        den += e_t1
        att = (num / den[None, :]).T.reshape(INNER)  # f = h*64 + d
        out[b, 0, :] = (att @ w_o + b_o).astype(np.float32)

        # tok1 row: attends {CLS, itself} only — fully host-computed
        e_c = np.exp(SCALE * (q_t1 * k_cls).sum(1))
        e_1 = np.exp(SCALE * (q_t1 * k_t1).sum(1))
        att1 = ((e_c[:, None] * v_cls + e_1[:, None] * v_t1)
                / (e_c + e_1)[:, None]).reshape(INNER)
        out[b, 1, :] = (att1 @ w_o + b_o).astype(np.float32)
    return out


# revision 17
# speedup vs baseline: 1.1679x; 1.0147x over previous
import os
import sys

for _p in ("/opt/trn_rl_repo", "/root/.axon_site/_ro/trn_rl_repo"):
    if os.path.isdir(_p) and _p not in sys.path:
        sys.path.insert(0, _p)

import numpy as np
import ml_dtypes

BF = ml_dtypes.bfloat16

HEADS, D = 12, 64
WINDOW, SHIFT = 16, 1
SCALE = D ** -0.5
B, N, DIM = 2, 2049, 768
INNER = HEADS * D  # 768
TAUG = 258  # CLS slot + tok1/dummy slot + 256 block tokens
NCORES = 8
KT = DIM // 128  # 6
VW = 65  # per-head v stride: 64 value cols + 1 ones col (denominator)

# global token ranges owned by each core (block attention); all starts == 2 mod 16
STARTS = [2, 258, 514, 770, 1026, 1282, 1538, 1794]
ENDS = [258, 514, 770, 1026, 1282, 1538, 1794, 2049]

_NC_CACHE = {}


def _build_nc():
    import concourse.bass as bass
    import concourse.bacc as bacc
    import concourse.mybir as mybir
    import concourse.tile as tile

    f32 = mybir.dt.float32
    bf16 = mybir.dt.bfloat16
    Exp = mybir.ActivationFunctionType.Exp
    Copy = mybir.ActivationFunctionType.Copy

    nc = bacc.Bacc(None, target_bir_lowering=False)

    xaT_ext = nc.declare_dram_parameter("xaT", (B, DIM, TAUG), bf16, isOutput=False)
    wqkv_ext = nc.declare_dram_parameter("w_qkv", (DIM, 3 * INNER), bf16, isOutput=False)
    wout_ext = nc.declare_dram_parameter("w_out", (INNER, DIM), bf16, isOutput=False)
    bout_ext = nc.declare_dram_parameter("b_out", (128, DIM), f32, isOutput=False)
    mask_ext = nc.declare_dram_parameter("masks", (128, 520), bf16, isOutput=False)
    mcls_ext = nc.declare_dram_parameter("mask_cls", (HEADS, TAUG), f32, isOutput=False)
    vspec_ext = nc.declare_dram_parameter("v_spec", (B, HEADS, HEADS * VW), bf16, isOutput=False)
    out_ext = nc.declare_dram_parameter("out_tokens", (B, 256, DIM), f32, isOutput=True)
    cls_ext = nc.declare_dram_parameter("cls_all", (B, VW, HEADS), f32, isOutput=True)

    with tile.TileContext(nc) as tc:
        with (
            tc.tile_pool(name="wpool", bufs=1) as wp,
            tc.tile_pool(name="fpool", bufs=2) as fp,
            tc.tile_pool(name="inpool", bufs=2) as ip,
            tc.tile_pool(name="spool", bufs=3) as sp,
            tc.tile_pool(name="psBig", bufs=2, space="PSUM") as psBig,
            tc.tile_pool(name="psSc", bufs=2, space="PSUM") as psSc,
            tc.tile_pool(name="psAv", bufs=2, space="PSUM") as psAv,
            tc.tile_pool(name="psO", bufs=2, space="PSUM") as psO,
        ):
            # ---- DMA order tuned: batch-0 x first, then w_qkv by column
            # ---- group (k cols first — they gate the head loop), wo/bias last
            # warm the PE clock with dummy matmuls that depend on nothing
            wmsb = wp.tile([128, 128], bf16, tag="wmsb")
            nc.vector.memset(wmsb[:], 0.25)
            wmps = psO.tile([128, 512], f32, tag="o", name="wmps")
            for _ in range(11):
                nc.tensor.matmul(wmps[:, 0:128], wmsb[:], wmsb[:, 0:128], start=True,
                                 stop=True, skip_group_check=True)
            Xw = [None] * B
            t = ip.tile([128, KT * TAUG], bf16, tag="xw", name="xW0")
            nc.sync.dma_start(t[:].rearrange("p (k t) -> p k t", t=TAUG),
                              xaT_ext[0].rearrange("(k p) t -> p k t", p=128))
            Xw[0] = t
            Xb = [[None] * KT for _ in range(B)]
            Xb[0] = [Xw[0][:, k * TAUG:(k + 1) * TAUG] for k in range(KT)]
            mask_t = wp.tile([128, 520], bf16, tag="mask")
            nc.sync.dma_start(mask_t[:], mask_ext[:])
            mclst = wp.tile([HEADS, TAUG], f32, tag="mcls")
            nc.sync.dma_start(mclst[:], mcls_ext[:])
            Ww = wp.tile([128, KT * 3 * INNER], bf16, tag="ww", name="Ww")
            W = [Ww[:, k * 3 * INNER:(k + 1) * 3 * INNER] for k in range(KT)]
            wsrc = wqkv_ext[:].rearrange("(k p) c -> p k c", p=128)
            wdst = Ww[:].rearrange("p (k c) -> p k c", c=3 * INNER)
            for grp in (1, 0, 2):  # k cols, q cols, v cols
                nc.sync.dma_start(wdst[:, :, grp * INNER:(grp + 1) * INNER],
                                  wsrc[:, :, grp * INNER:(grp + 1) * INNER])
            t = ip.tile([128, KT * TAUG], bf16, tag="xw", name="xW1")
            nc.sync.dma_start(t[:].rearrange("p (k t) -> p k t", t=TAUG),
                              xaT_ext[1].rearrange("(k p) t -> p k t", p=128))
            Xw[1] = t
            Xb[1] = [Xw[1][:, k * TAUG:(k + 1) * TAUG] for k in range(KT)]
            bias_t = wp.tile([128, DIM], f32, tag="bias")
            nc.sync.dma_start(bias_t[:], bout_ext[:])
            WOw = wp.tile([128, KT * DIM], bf16, tag="wow", name="WOw")
            nc.sync.dma_start(WOw[:].rearrange("p (k c) -> p k c", c=DIM),
                              wout_ext[:].rearrange("(k p) c -> p k c", p=128))
            WO = [WOw[:, k * DIM:(k + 1) * DIM] for k in range(KT)]

            def proj_phase(b):
                X = Xb[b]
                vspec_t = ip.tile([HEADS, HEADS * VW], bf16, tag="vspec", name=f"vspec{b}")
                nc.sync.dma_start(vspec_t[:], vspec_ext[b])

                QK = [None] * 12
                for fo in [6, 7, 8, 9, 10, 11, 0, 1, 2, 3, 4, 5]:
                    P = psBig.tile([128, TAUG], f32, tag="big")
                    for fi in range(KT):
                        nc.tensor.matmul(P[:], W[fi][:, fo * 128:(fo + 1) * 128], X[fi][:],
                                         start=(fi == 0), stop=(fi == KT - 1))
                    t = fp.tile([128, TAUG], bf16, tag=f"qk{fo}", name=f"qk{b}_{fo}")
                    nc.vector.tensor_copy(t[:], P[:])
                    QK[fo] = t

                kbd = fp.tile([128, KT * HEADS], bf16, tag="kbd", name=f"kbd{b}")
                nc.vector.memset(kbd[:], 0.0)
                for h in range(HEADS):
                    r0 = (h % 2) * 64
                    c = (h // 2) * HEADS + h
                    nc.vector.tensor_copy(kbd[r0:r0 + 64, c:c + 1],
                                          QK[6 + h // 2][r0:r0 + 64, 0:1])
                E = psAv.tile([HEADS, TAUG], f32, tag="av", name=f"E{b}")
                for i in range(KT):
                    nc.tensor.matmul(E[:], kbd[:, i * HEADS:(i + 1) * HEADS], QK[i][:],
                                     start=(i == 0), stop=(i == KT - 1))
                esb = sp.tile([HEADS, TAUG], f32, tag="esb", name=f"esb{b}")
                nc.scalar.activation(esb[:], E[:], Exp, scale=SCALE)
                esb_bf = fp.tile([HEADS, TAUG], bf16, tag="esb_bf", name=f"esbbf{b}")
                nc.vector.tensor_mul(esb_bf[:], esb[:], mclst[:])

                Vsb = []
                for s in range(2):
                    vt = fp.tile([128, HEADS * VW], bf16, tag=f"v{s}", name=f"v{b}_{s}")
                    for half in range(2):
                        c0, cw, nh = (0, 512, 8) if half == 0 else (512, 256, 4)
                        P = psBig.tile([128, cw], f32, tag="big")
                        for fi in range(KT):
                            nc.tensor.matmul(P[:], X[fi][:, 2 + 128 * s: 2 + 128 * (s + 1)],
                                             W[fi][:, 2 * INNER + c0: 2 * INNER + c0 + cw],
                                             start=(fi == 0), stop=(fi == KT - 1))
                        dst = vt[:, half * 8 * VW: half * 8 * VW + nh * VW]
                        dst = dst.rearrange("p (h c) -> p h c", c=VW)[:, :, 0:64]
                        src2 = P[:].rearrange("p (h c) -> p h c", c=64)
                        nc.vector.tensor_copy(dst, src2)
                    nc.vector.memset(vt[:, 64:HEADS * VW:VW], 1.0)
                    Vsb.append(vt)

                aTfm = [fp.tile([128, TAUG], bf16, tag=f"a{i}", name=f"aTfm{b}_{i}")
                        for i in range(KT)]
                cls_all = sp.tile([VW, HEADS], f32, tag="cls_all", name=f"cls{b}")
                return dict(vspec=vspec_t, QK=QK, esb_bf=esb_bf, Vsb=Vsb,
                            aTfm=aTfm, cls_all=cls_all, O=[None, None])

            def head_pair(b, st, j):
                QK, Vsb, esb_bf, vspec_t = st["QK"], st["Vsb"], st["esb_bf"], st["vspec"]
                aTfm, cls_all = st["aTfm"], st["cls_all"]
                ha, hb2 = 2 * j, 2 * j + 1
                qa, ka = QK[j][0:64, :], QK[6 + j][0:64, :]
                qb, kb = QK[j][64:128, :], QK[6 + j][64:128, :]

                Ba = psAv.tile([VW, TAUG], f32, tag="av", name=f"Ba{b}_{j}")
                Bb = psAv.tile([VW, TAUG], f32, tag="av", name=f"Bb{b}_{j}")
                nc.tensor.matmul(Ba[:], vspec_t[:, ha * VW:(ha + 1) * VW], esb_bf[:],
                                 start=True, stop=False)
                nc.tensor.matmul(Bb[:], vspec_t[:, hb2 * VW:(hb2 + 1) * VW], esb_bf[:],
                                 start=True, stop=False)
                Aa = psSc.tile([128, 260], f32, tag="sc", name=f"Aa{b}_{j}")
                Ab = psSc.tile([128, 260], f32, tag="sc", name=f"Ab{b}_{j}")
                nc.tensor.matmul(Aa[:, 0:130], ka[:, 2:130], qa[:, 0:130],
                                 start=True, stop=True)
                nc.tensor.matmul(Ab[:, 0:130], kb[:, 2:130], qb[:, 0:130],
                                 start=True, stop=True)
                nc.tensor.matmul(Aa[:, 130:258], ka[:, 130:258], qa[:, 130:258],
                                 start=True, stop=True, skip_group_check=True)
                nc.tensor.matmul(Ab[:, 130:258], kb[:, 130:258], qb[:, 130:258],
                                 start=True, stop=True, skip_group_check=True)
                nc.tensor.matmul(Aa[:, 258:259], ka[:, 130:258], qa[:, 0:1],
                                 start=True, stop=True, skip_group_check=True)
                nc.tensor.matmul(Ab[:, 258:259], kb[:, 130:258], qb[:, 0:1],
                                 start=True, stop=True, skip_group_check=True)
                proba = sp.tile([128, 260], bf16, tag="proba")
                probb = sp.tile([128, 260], bf16, tag="probb")
                nc.scalar.activation(proba[:, 0:259], Aa[:, 0:259], Exp, scale=SCALE)
                nc.scalar.activation(probb[:, 0:259], Ab[:, 0:259], Exp, scale=SCALE)
                nc.vector.tensor_mul(proba[:, 0:259], proba[:, 0:259], mask_t[:, 0:259])
                nc.vector.tensor_mul(probb[:, 0:259], probb[:, 0:259], mask_t[:, 0:259])
                for Bx, px, h in ((Ba, proba, ha), (Bb, probb, hb2)):
                    nc.tensor.matmul(Bx[:, 0:130], Vsb[0][:, h * VW:(h + 1) * VW],
                                     px[:, 0:130], start=False, stop=False)
                    nc.tensor.matmul(Bx[:, 130:258], Vsb[1][:, h * VW:(h + 1) * VW],
                                     px[:, 130:258], start=False, stop=False)
                    nc.tensor.matmul(Bx[:, 0:1], Vsb[1][:, h * VW:(h + 1) * VW],
                                     px[:, 258:259], start=False, stop=True)
                for Bx, h, r0 in ((Ba, ha, 0), (Bb, hb2, 64)):
                    dsb = sp.tile([1, TAUG], f32, tag="dsb")
                    nc.scalar.activation(dsb[:], Bx[64:65, :], Copy)
                    rec = sp.tile([1, TAUG], f32, tag="rec")
                    nc.vector.reciprocal_approx_fast(rec[:], dsb[:])
                    recb = sp.tile([D, TAUG], f32, tag="recb")
                    nc.gpsimd.partition_broadcast(recb[:], rec[:])
                    nc.vector.tensor_mul(aTfm[j][r0:r0 + 64, :], Bx[0:64, :], recb[:])
                    nc.vector.tensor_copy(cls_all[:, h:h + 1], Bx[:, 0:1])
                # out-proj tok-tile 0 accumulates feature tile j as it completes
                if j == 0:
                    st["O"][0] = psO.tile([128, 512], f32, tag="o", name=f"O0_{b}")
                    st["O"][1] = psO.tile([128, 256], f32, tag="o", name=f"O1_{b}")
                nc.tensor.matmul(st["O"][0][:], aTfm[j][:, 2:130], WO[j][:, 0:512],
                                 start=(j == 0), stop=(j == KT - 1))
                nc.tensor.matmul(st["O"][1][:], aTfm[j][:, 2:130], WO[j][:, 512:768],
                                 start=(j == 0), stop=(j == KT - 1))

            def finish_phase(b, st):
                nc.sync.dma_start(cls_ext[b], st["cls_all"][:])
                aTfm = st["aTfm"]
                osb0 = ip.tile([128, DIM], f32, tag="osb0", name=f"osb0_{b}")
                nc.vector.tensor_add(osb0[:, 0:512], st["O"][0][:], bias_t[:, 0:512])
                nc.vector.tensor_add(osb0[:, 512:768], st["O"][1][:], bias_t[:, 512:768])
                nc.sync.dma_start(out_ext[b, 0:128, :], osb0[:])
                osb1 = ip.tile([128, DIM], f32, tag="osb1", name=f"osb1_{b}")
                if "O1t" in st:
                    nc.vector.tensor_add(osb1[:, 0:512], st["O1t"][0][:], bias_t[:, 0:512])
                    nc.vector.tensor_add(osb1[:, 512:768], st["O1t"][1][:], bias_t[:, 512:768])
                else:
                    for half in range(2):
                        f0, fw = (0, 512) if half == 0 else (512, 256)
                        PO = psO.tile([128, fw], f32, tag="o", name=f"PO{b}_{half}")
                        for i in range(KT):
                            nc.tensor.matmul(PO[:], aTfm[i][:, 130:258], WO[i][:, f0:f0 + fw],
                                             start=(i == 0), stop=(i == KT - 1))
                        nc.vector.tensor_add(osb1[:, f0:f0 + fw], PO[:], bias_t[:, f0:f0 + fw])
                nc.sync.dma_start(out_ext[b, 128:256, :], osb1[:])

            for b in range(B):
                st = proj_phase(b)
                for j in range(KT):
                    head_pair(b, st, j)
                finish_phase(b, st)

    nc.compile()
    return nc


def _get_nc():
    if "nc" not in _NC_CACHE:
        _NC_CACHE["nc"] = _build_nc()
    return _NC_CACHE["nc"]


def _make_masks(core):
    start = STARTS[core]
    m = np.zeros((128, 260), dtype=np.float32)
    _fill_mask(m, core)
    return np.concatenate([m, m], axis=1)  # [128, 520] for the pair-merged mul


def _fill_mask(m, core):
    start = STARTS[core]
    for s in range(2):
        g = start + s * 128 + np.arange(128)  # global token ids of this stripe's keys
        real = g < 2049
        blk = (g - 2) // 16
        bd = (blk[:, None] == blk[None, :]) & real[:, None] & real[None, :]
        if s == 0:
            m[:, 0] = real.astype(np.float32)  # CLS query attends all real s0 keys
            m[:, 2:130] = bd
        else:
            m[:, 130:258] = bd
            m[:, 258] = real.astype(np.float32)  # CLS query col vs s1 keys


def make_in_maps(x, w_qkv, w_out, b_out):
    x = np.asarray(x, dtype=np.float32)
    w_qkv = np.asarray(w_qkv, dtype=np.float32)
    w_out = np.asarray(w_out, dtype=np.float32)
    b_out = np.asarray(b_out, dtype=np.float32)
    w_v = w_qkv[:, 2 * INNER:]
    wqkv_bf = w_qkv.astype(BF)
    wout_bf = w_out.astype(BF)
    bias128 = np.tile(b_out.reshape(1, DIM), (128, 1)).astype(np.float32)
    in_maps = []
    for c in range(NCORES):
        xa = np.zeros((B, TAUG, DIM), dtype=np.float32)
        xa[:, 0, :] = x[:, 0, :]
        if c == 0:
            xa[:, 1, :] = x[:, 1, :]
        L = ENDS[c] - STARTS[c]
        xa[:, 2:2 + L, :] = x[:, STARTS[c]:ENDS[c], :]
        xaT = np.ascontiguousarray(xa.transpose(0, 2, 1)).astype(BF)
        mask_cls = np.ones((HEADS, TAUG), dtype=np.float32)
        if c > 0:
            mask_cls[:, 0] = 0.0
        v_cls = xa[:, 0, :] @ w_v  # [B, 768]
        v_spec = np.zeros((B, HEADS, HEADS * VW), dtype=np.float32)
        for h in range(HEADS):
            v_spec[:, h, h * VW:h * VW + 64] = v_cls[:, h * 64:(h + 1) * 64]
            v_spec[:, h, h * VW + 64] = 1.0
        in_maps.append({
            "xaT": xaT,
            "w_qkv": wqkv_bf,
            "w_out": wout_bf,
            "b_out": bias128,
            "masks": _make_masks(c).astype(BF),
            "mask_cls": mask_cls,
            "v_spec": v_spec.astype(BF),
        })
    return in_maps


def kernel(x, w_qkv, w_out, b_out):
    x = np.asarray(x, dtype=np.float32)
    w_qkv = np.asarray(w_qkv, dtype=np.float32)
    w_out = np.asarray(w_out, dtype=np.float32)
    b_out = np.asarray(b_out, dtype=np.float32)

    in_maps = make_in_maps(x, w_qkv, w_out, b_out)

    from concourse.bass_utils import run_bass_kernel_spmd

    nc = _get_nc()
    res = run_bass_kernel_spmd(nc, in_maps, core_ids=list(range(NCORES))).results

    out = np.empty((B, N, DIM), dtype=np.float32)
    for c in range(NCORES):
        L = ENDS[c] - STARTS[c]
        out[:, STARTS[c]:ENDS[c], :] = res[c]["out_tokens"][:, :L, :]

    # host-side special rows (CLS = global token 0, tok1 = global token 1)
    w_q = w_qkv[:, :INNER].astype(np.float64)
    w_k = w_qkv[:, INNER:2 * INNER].astype(np.float64)
    w_v = w_qkv[:, 2 * INNER:].astype(np.float64)
    w_o = w_out.astype(np.float64)
    b_o = b_out.astype(np.float64)
    for b in range(B):
        x0 = x[b, 0].astype(np.float64)
        x1 = x[b, 1].astype(np.float64)
        q_cls = (x0 @ w_q).reshape(HEADS, D)
        q_t1 = (x1 @ w_q).reshape(HEADS, D)
        k_cls = (x0 @ w_k).reshape(HEADS, D)
        k_t1 = (x1 @ w_k).reshape(HEADS, D)
        v_cls = (x0 @ w_v).reshape(HEADS, D)
        v_t1 = (x1 @ w_v).reshape(HEADS, D)

        # CLS row: sum device partials over cores, add tok1-as-key term on host
        num = np.zeros((D, HEADS), dtype=np.float64)
        den = np.zeros(HEADS, dtype=np.float64)
        for c in range(NCORES):
            cl = res[c]["cls_all"][b].astype(np.float64)
            num += cl[0:D, :]
            den += cl[D, :]
        e_t1 = np.exp(SCALE * (q_cls * k_t1).sum(1))  # [HEADS]
        num += v_t1.T * e_t1[None, :]
        den += e_t1
        att = (num / den[None, :]).T.reshape(INNER)  # f = h*64 + d
        out[b, 0, :] = (att @ w_o + b_o).astype(np.float32)

        # tok1 row: attends {CLS, itself} only — fully host-computed
        e_c = np.exp(SCALE * (q_t1 * k_cls).sum(1))
        e_1 = np.exp(SCALE * (q_t1 * k_t1).sum(1))
        att1 = ((e_c[:, None] * v_cls + e_1[:, None] * v_t1)
                / (e_c + e_1)[:, None]).reshape(INNER)
        out[b, 1, :] = (att1 @ w_o + b_o).astype(np.float32)
    return out
